# revision 19
# baseline (speedup 1.0000x reference)
"""AdaBIGGAN adaptive 1x1-conv stage, data-parallel across 8 TRN2 NeuronCores.

Math (per sample b):
    scale[b, c] = sum_k y[b, k] * Wsum[c, k] + bsum[c]
        where Wsum[c, k] = sum_j Wg_w[c*C + j, k],  bsum[c] = sum_j Wg_b[c*C + j]
    bias[b, c]  = sum_k y[b, k] * Bg_w[c, k] + Bg_b[c]
    out[b, c, :, :] = relu(h[b, c, :, :] * scale[b, c] + bias[b, c])

Sharding: batch B=32 split 4-per-core across 8 cores; hypernet params replicated.
"""

import numpy as np

import concourse.bacc as bacc
import concourse.mybir as mybir
from concourse.tile import TileContext
from concourse.bass_utils import run_bass_kernel_spmd

_B, _C, _H, _W, _IN = 32, 96, 128, 128, 148
_NCORES = 8
_BL = _B // _NCORES          # 4 samples per core
_HW = _H * _W                # 16384
_ROWS = _BL * _C             # 384 rows = 3 x 128 partitions
_NPT = 3                     # row tiles of 128
_FCH = 4096                  # free-dim chunk of the h stream
_WGC = 8                     # Wg_w load chunks (along j)
_JC = _C // _WGC             # j per chunk = 12
_F32 = mybir.dt.float32

LAST_RESULTS = None


def _build():
    nc = bacc.Bacc(None)
    h = nc.declare_dram_parameter("h", [_ROWS, _HW], _F32, isOutput=False)
    yb = nc.declare_dram_parameter("yb", [_C, _BL * _IN], _F32, isOutput=False)
    wg = nc.declare_dram_parameter("wg", [_C, _JC * _IN], _F32, isOutput=False)
    wb = nc.declare_dram_parameter("wb", [_C, _C], _F32, isOutput=False)
    bw = nc.declare_dram_parameter("bw", [_C, _IN], _F32, isOutput=False)
    bb = nc.declare_dram_parameter("bb", [_C, 1], _F32, isOutput=False)
    out = nc.declare_dram_parameter("out", [_ROWS, _HW], _F32, isOutput=True)

    with TileContext(nc) as tc:
        with (
            tc.tile_pool(name="hyper", bufs=1) as hp,
            tc.tile_pool(name="stream", bufs=6) as sp,
            tc.tile_pool(name="ccdram", bufs=1, space="DRAM") as dp,
        ):
            # --- hypernetwork: per-(b,c) scale/bias scalars -------------------
            # hyper loads ride the scalar HWDGE ring: it is idle early (stores
            # only start once scale/bias are ready), so the h stream on the
            # sync ring never blocks behind them.
            wb_t = hp.tile([_C, _C], _F32)         # [c, j]
            nc.scalar.dma_start(out=wb_t[:], in_=wb[:])
            bw_t = hp.tile([_C, _IN], _F32)        # [c, k]
            nc.scalar.dma_start(out=bw_t[:], in_=bw[:])
            bb_t = hp.tile([_C, 1], _F32)          # [c]
            nc.scalar.dma_start(out=bb_t[:], in_=bb[:])
            y_t = hp.tile([_C, _BL * _IN], _F32)   # y broadcast to all c rows
            nc.scalar.dma_start(out=y_t[:], in_=yb[:])

            # Wg_w row-sum is sharded: each core loads only its j-block
            # (Wg_w[:, 12*i:12*(i+1), :] flattened, 0.68 MB), computes a
            # partial Wsum, and the 57 KB partials are AllReduce'd.
            acc = hp.tile([_C, _JC * _IN], _F32)         # [c, (12 148)]
            nc.scalar.dma_start(out=acc[:], in_=wg[:])
            nc.vector.tensor_add(acc[:, :6 * _IN], acc[:, :6 * _IN],
                                 acc[:, 6 * _IN:12 * _IN])
            nc.vector.tensor_add(acc[:, :3 * _IN], acc[:, :3 * _IN],
                                 acc[:, 3 * _IN:6 * _IN])
            part = hp.tile([_C, _IN], _F32)
            nc.vector.tensor_reduce(
                out=part[:],
                in_=acc[:, :3 * _IN].rearrange("c (j k) -> c k j", j=3, k=_IN),
                axis=mybir.AxisListType.X,
                op=mybir.AluOpType.add,
            )
            cc_in = dp.tile([_C, _IN], _F32)
            cc_out = dp.tile([_C, _IN], _F32, addr_space="Shared")
            nc.scalar.dma_start(out=cc_in[:], in_=part[:])
            nc.gpsimd.collective_compute(
                "AllReduce",
                mybir.AluOpType.add,
                replica_groups=[list(range(_NCORES))],
                ins=[cc_in[:]],
                outs=[cc_out[:]],
            )
            wsum = hp.tile([_C, _IN], _F32)
            nc.scalar.dma_start(out=wsum[:], in_=cc_out[:])

            # bsum[c] = sum_j Wg_b[(c j)]
            bsum = hp.tile([_C, 1], _F32)
            nc.vector.tensor_reduce(
                out=bsum[:], in_=wb_t[:],
                axis=mybir.AxisListType.X, op=mybir.AluOpType.add,
            )

            scale_t = hp.tile([_C, _BL], _F32)     # scale^T: [c, b]
            bias_t = hp.tile([_C, _BL], _F32)      # bias^T:  [c, b]
            junk = hp.tile([_C, _IN], _F32)
            junk2 = hp.tile([_C, _IN], _F32)
            for b in range(_BL):
                yb_ap = y_t[:, b * _IN:(b + 1) * _IN]
                nc.vector.tensor_mul(junk[:], wsum[:], yb_ap)
                nc.vector.tensor_reduce(
                    out=scale_t[:, b:b + 1], in_=junk[:],
                    axis=mybir.AxisListType.X, op=mybir.AluOpType.add,
                )
                nc.vector.tensor_mul(junk2[:], bw_t[:], yb_ap)
                nc.vector.tensor_reduce(
                    out=bias_t[:, b:b + 1], in_=junk2[:],
                    axis=mybir.AxisListType.X, op=mybir.AluOpType.add,
                )
            nc.vector.tensor_scalar_add(scale_t[:], scale_t[:], bsum[:])
            nc.vector.tensor_scalar_add(bias_t[:], bias_t[:], bb_t[:])

            # Re-lay [c, b] -> flat [b*C + c] as 3 x [128, 2] tiles (col 0 =
            # scale, col 1 = bias) with direct SBUF->SBUF partition-range
            # copies, split at batch boundaries.
            sb_fl = []
            for r in range(_NPT):
                t = hp.tile([128, 2], _F32, tag=f"fl{r}")
                p = 0
                f = r * 128
                while p < 128:
                    b, c = (f + p) // _C, (f + p) % _C
                    n = min(128 - p, _C - c)
                    nc.scalar.dma_start(out=t[p:p + n, 0:1],
                                        in_=scale_t[c:c + n, b:b + 1])
                    nc.scalar.dma_start(out=t[p:p + n, 1:2],
                                        in_=bias_t[c:c + n, b:b + 1])
                    p += n
                sb_fl.append(t)

            # --- stream h: out = relu(h * scale + bias), fused in ScalarE ----
            # loads on sync HWDGE ring, stores on scalar HWDGE ring
            for r in range(_NPT):
                rows = slice(r * 128, (r + 1) * 128)
                for f0 in range(0, _HW, _FCH):
                    t = sp.tile([128, _FCH], _F32)
                    nc.sync.dma_start(out=t[:], in_=h[rows, f0:f0 + _FCH])
                    nc.scalar.activation(
                        out=t[:], in_=t[:],
                        func=mybir.ActivationFunctionType.Relu,
                        bias=sb_fl[r][:, 1:2],
                        scale=sb_fl[r][:, 0:1],
                    )
                    nc.scalar.dma_start(out=out[rows, f0:f0 + _FCH], in_=t[:])
    nc.finalize()
    return nc


def kernel(h, y, Wg_w, Wg_b, Bg_w, Bg_b):
    global LAST_RESULTS
    h = np.ascontiguousarray(h, np.float32)
    y = np.ascontiguousarray(y, np.float32)

    nc = _build()
    wg_3 = np.ascontiguousarray(Wg_w, np.float32).reshape(_C, _C, _IN)
    wb_r = np.ascontiguousarray(Wg_b, np.float32).reshape(_C, _C)
    bw_r = np.ascontiguousarray(Bg_w, np.float32)
    bb_r = np.ascontiguousarray(Bg_b, np.float32).reshape(_C, 1)

    in_maps = []
    for i in range(_NCORES):
        hs = h[i * _BL:(i + 1) * _BL].reshape(_ROWS, _HW)
        ys = y[i * _BL:(i + 1) * _BL].reshape(1, _BL * _IN)
        wg_i = np.ascontiguousarray(
            wg_3[:, i * _JC:(i + 1) * _JC, :]).reshape(_C, _JC * _IN)
        in_maps.append({
            "h": np.ascontiguousarray(hs),
            "yb": np.ascontiguousarray(np.broadcast_to(ys, (_C, _BL * _IN))),
            "wg": wg_i, "wb": wb_r, "bw": bw_r, "bb": bb_r,
        })

    res = run_bass_kernel_spmd(nc, in_maps, core_ids=list(range(_NCORES)))
    LAST_RESULTS = res
    outs = [r["out"].reshape(_BL, _C, _H, _W) for r in res.results]
    return np.concatenate(outs, axis=0)


# revision 24
# speedup vs baseline: 1.2509x; 1.2509x over previous
"""AdaBIGGAN adaptive 1x1-conv stage, data-parallel across 8 TRN2 NeuronCores.

Math (per sample b):
    scale[b, c] = sum_k y[b, k] * Wsum[c, k] + bsum[c]
        where Wsum[c, k] = sum_j Wg_w[c*C + j, k],  bsum[c] = sum_j Wg_b[c*C + j]
    bias[b, c]  = sum_k y[b, k] * Bg_w[c, k] + Bg_b[c]
    out[b, c, :, :] = relu(h[b, c, :, :] * scale[b, c] + bias[b, c])

Sharding: batch B=32 split 4-per-core across 8 cores; hypernet params replicated.
"""

import numpy as np

import concourse.bacc as bacc
import concourse.mybir as mybir
from concourse.tile import TileContext
from concourse.bass_utils import run_bass_kernel_spmd

_B, _C, _H, _W, _IN = 32, 96, 128, 128, 148
_NCORES = 8
_BL = _B // _NCORES          # 4 samples per core
_HW = _H * _W                # 16384
_ROWS = _BL * _C             # 384 rows = 3 x 128 partitions
_NPT = 3                     # row tiles of 128
_FCH = 4096                  # free-dim chunk of the h stream
_WGC = 8                     # Wg_w load chunks (along j)
_JC = _C // _WGC             # j per chunk = 12
_F32 = mybir.dt.float32

LAST_RESULTS = None


def _build():
    nc = bacc.Bacc(None)
    h = nc.declare_dram_parameter("h", [_ROWS, _HW], _F32, isOutput=False)
    yb = nc.declare_dram_parameter("yb", [_C, _BL * _IN], _F32, isOutput=False)
    wg = nc.declare_dram_parameter("wg", [_C, _C * _IN], _F32, isOutput=False)
    wb = nc.declare_dram_parameter("wb", [_C, _C], _F32, isOutput=False)
    bw = nc.declare_dram_parameter("bw", [_C, _IN], _F32, isOutput=False)
    bb = nc.declare_dram_parameter("bb", [_C, 1], _F32, isOutput=False)
    out = nc.declare_dram_parameter("out", [_ROWS, _HW], _F32, isOutput=True)

    with TileContext(nc) as tc:
        with (
            tc.tile_pool(name="hyper", bufs=1) as hp,
            tc.tile_pool(name="stream", bufs=6) as sp,
        ):
            # --- hypernetwork: per-(b,c) scale/bias scalars -------------------
            # hyper loads ride the scalar HWDGE ring: it is idle early (stores
            # only start once scale/bias are ready), so the h stream on the
            # sync ring never blocks behind them.
            wb_t = hp.tile([_C, _C], _F32)         # [c, j]
            nc.scalar.dma_start(out=wb_t[:], in_=wb[:])
            bw_t = hp.tile([_C, _IN], _F32)        # [c, k]
            nc.scalar.dma_start(out=bw_t[:], in_=bw[:])
            bb_t = hp.tile([_C, 1], _F32)          # [c]
            nc.scalar.dma_start(out=bb_t[:], in_=bb[:])
            y_t = hp.tile([_C, _BL * _IN], _F32)   # y broadcast to all c rows
            nc.scalar.dma_start(out=y_t[:], in_=yb[:])

            # Wg_w loaded in _WGC chunks along j, split across BOTH HWDGE
            # rings ahead of the h stream so they land in ~13 us. A serial
            # DVE accumulate folds chunks as they arrive; j then halved
            # 12 -> 6 -> 3 and a small strided reduce finishes Wsum.
            chunks = []
            for m in range(_WGC):
                wg_m = hp.tile([_C, _JC * _IN], _F32, tag=f"wg{m}")
                eng = nc.sync if m % 2 == 0 else nc.scalar
                eng.dma_start(
                    out=wg_m[:], in_=wg[:, m * _JC * _IN:(m + 1) * _JC * _IN])
                chunks.append(wg_m)
            acc = chunks[0]
            for m in range(1, _WGC):
                nc.vector.tensor_add(acc[:], acc[:], chunks[m][:])
            nc.vector.tensor_add(acc[:, :6 * _IN], acc[:, :6 * _IN],
                                 acc[:, 6 * _IN:12 * _IN])
            nc.vector.tensor_add(acc[:, :3 * _IN], acc[:, :3 * _IN],
                                 acc[:, 3 * _IN:6 * _IN])
            wsum = hp.tile([_C, _IN], _F32)
            nc.vector.tensor_reduce(
                out=wsum[:],
                in_=acc[:, :3 * _IN].rearrange("c (j k) -> c k j", j=3, k=_IN),
                axis=mybir.AxisListType.X,
                op=mybir.AluOpType.add,
            )

            # bsum[c] = sum_j Wg_b[(c j)]
            bsum = hp.tile([_C, 1], _F32)
            nc.vector.tensor_reduce(
                out=bsum[:], in_=wb_t[:],
                axis=mybir.AxisListType.X, op=mybir.AluOpType.add,
            )

            scale_t = hp.tile([_C, _BL], _F32)     # scale^T: [c, b]
            bias_t = hp.tile([_C, _BL], _F32)      # bias^T:  [c, b]
            junk = hp.tile([_C, _IN], _F32)
            junk2 = hp.tile([_C, _IN], _F32)
            for b in range(_BL):
                yb_ap = y_t[:, b * _IN:(b + 1) * _IN]
                nc.vector.tensor_mul(junk[:], wsum[:], yb_ap)
                nc.vector.tensor_reduce(
                    out=scale_t[:, b:b + 1], in_=junk[:],
                    axis=mybir.AxisListType.X, op=mybir.AluOpType.add,
                )
                nc.vector.tensor_mul(junk2[:], bw_t[:], yb_ap)
                nc.vector.tensor_reduce(
                    out=bias_t[:, b:b + 1], in_=junk2[:],
                    axis=mybir.AxisListType.X, op=mybir.AluOpType.add,
                )
            nc.vector.tensor_scalar_add(scale_t[:], scale_t[:], bsum[:])
            nc.vector.tensor_scalar_add(bias_t[:], bias_t[:], bb_t[:])

            # Re-lay [c, b] -> flat [b*C + c] as 3 x [128, 2] tiles (col 0 =
            # scale, col 1 = bias) with direct SBUF->SBUF partition-range
            # copies, split at batch boundaries.
            sb_fl = []
            for r in range(_NPT):
                t = hp.tile([128, 2], _F32, tag=f"fl{r}")
                p = 0
                f = r * 128
                while p < 128:
                    b, c = (f + p) // _C, (f + p) % _C
                    n = min(128 - p, _C - c)
                    nc.scalar.dma_start(out=t[p:p + n, 0:1],
                                        in_=scale_t[c:c + n, b:b + 1])
                    nc.scalar.dma_start(out=t[p:p + n, 1:2],
                                        in_=bias_t[c:c + n, b:b + 1])
                    p += n
                sb_fl.append(t)

            # --- stream h: out = relu(h * scale + bias), fused in ScalarE ----
            # loads on sync HWDGE ring, stores on scalar HWDGE ring
            for r in range(_NPT):
                rows = slice(r * 128, (r + 1) * 128)
                for f0 in range(0, _HW, _FCH):
                    t = sp.tile([128, _FCH], _F32)
                    nc.sync.dma_start(out=t[:], in_=h[rows, f0:f0 + _FCH])
                    nc.scalar.activation(
                        out=t[:], in_=t[:],
                        func=mybir.ActivationFunctionType.Relu,
                        bias=sb_fl[r][:, 1:2],
                        scale=sb_fl[r][:, 0:1],
                    )
                    nc.scalar.dma_start(out=out[rows, f0:f0 + _FCH], in_=t[:])
    nc.finalize()
    return nc


def kernel(h, y, Wg_w, Wg_b, Bg_w, Bg_b):
    global LAST_RESULTS
    h = np.ascontiguousarray(h, np.float32)
    y = np.ascontiguousarray(y, np.float32)

    nc = _build()
    wg_r = np.ascontiguousarray(Wg_w, np.float32).reshape(_C, _C * _IN)
    wb_r = np.ascontiguousarray(Wg_b, np.float32).reshape(_C, _C)
    bw_r = np.ascontiguousarray(Bg_w, np.float32)
    bb_r = np.ascontiguousarray(Bg_b, np.float32).reshape(_C, 1)

    in_maps = []
    for i in range(_NCORES):
        hs = h[i * _BL:(i + 1) * _BL].reshape(_ROWS, _HW)
        ys = y[i * _BL:(i + 1) * _BL].reshape(1, _BL * _IN)
        in_maps.append({
            "h": np.ascontiguousarray(hs),
            "yb": np.ascontiguousarray(np.broadcast_to(ys, (_C, _BL * _IN))),
            "wg": wg_r, "wb": wb_r, "bw": bw_r, "bb": bb_r,
        })

    res = run_bass_kernel_spmd(nc, in_maps, core_ids=list(range(_NCORES)))
    LAST_RESULTS = res
    outs = [r["out"].reshape(_BL, _C, _H, _W) for r in res.results]
    return np.concatenate(outs, axis=0)
